# revision 5
# baseline (speedup 1.0000x reference)
"""Trainium2 Bass kernel for GNN mean aggregation (nn_AggrGSMean), v2.

Computes, for t in {0,1}:
    out_t[b, v, :] = segment_sum(features_t over edges with dest v) / degree[b, v, t]
where degree[b, v, t] = max(count(adjacency[b, v, t, :] >= 0), 1).

Strategy (graph-partition sharding; identity-weight redesign):
- Host: partition vertices across 8 cores; per (core, table) sort vertices by
  pair count (pairs = ceil(edges/2)) descending and chunk into 98 blocks of
  128.  Every vertex in block b is padded to exactly profile[b] pairs
  (profile = max tile count over cores x tables at each block rank, ~2%
  padding), so tile i of a block holds pair i of each of its 128 vertices in
  lane order: the matmul's stationary operand is a CONSTANT IDENTITY - no
  one-hot building on device at all.
- Features ship as fp8 e4m3 with 4/degree folded in, quantized with an
  error-feedback cascade per (vertex, column): each edge stores
  fp8(x + carry), so the device's exact f32 segment-sum recovers the true
  sum to within one final-carry (~0.7% rel err).  Host multiplies the bf16
  output by 0.25 (exact) during assembly.
- Device (per core): per table, feature stream [128, TOTW] fp8 arrives in
  ~8-block chunk DMAs (~1-2 MB each, alternating SP/ACT/GPSIMD rings).  Per
  block: DoubleRow fp8 matmuls (identity weights, groups of 8 tiles, N=512)
  accumulate pair-rows into a [128, 512] f32 PSUM bank at 2 fp8 MACs per
  cell per cycle; even remainders use a narrower DoubleRow, a final odd tile
  uses a normal-mode matmul (FWL).  One DVE tensor_reduce folds the psum
  column-groups + q-halves into the bf16 group output tile; out DMA per 7
  blocks.
"""

import sys

if "/opt/trn_rl_repo" not in sys.path:
    sys.path.insert(0, "/opt/trn_rl_repo")

import ml_dtypes
import numpy as np

# Problem constants (hardcoded per contract)
B, V, T, N, F, M = 1, 100000, 2, 32, 64, 1600000
NCORES = 8
BLK_V = 128                 # vertices per block == matmul output partitions
Q = 2                       # edges per pair-row
ROW_W = Q * F               # fp8 bytes per pair-row (128)
G = 7                       # blocks per output group
VLOC = V // NCORES          # 12500
VPAD = 12544                # padded vertices per core (98 * 128)
NBLK = VPAD // BLK_V        # 98
NGRP = NBLK // G            # 14
CHUNK = 4                   # blocks per feature DMA
SCALE = 4.0                 # power-of-2 pre-scale (undone exactly on host)

FP8 = ml_dtypes.float8_e4m3

_NC_CACHE = {}


def shard_table(indices):
    """Sort edges by destination; assign vertices to (block, lane) per core
    by descending pair count."""
    v = np.ascontiguousarray(indices[:, 1]).astype(np.int64)
    order = np.argsort(v, kind="stable")
    vs = v[order]
    n_v = np.bincount(vs, minlength=V)
    starts = np.concatenate([[0], np.cumsum(n_v)])
    r = np.arange(len(vs), dtype=np.int64) - starts[vs]
    pv = (n_v + 1) // 2

    pvc = np.zeros((NCORES, VPAD), dtype=np.int64)
    pvc[:, :VLOC] = pv.reshape(NCORES, VLOC)
    slot_of = np.empty((NCORES, VPAD), dtype=np.int64)
    tb = np.empty((NCORES, NBLK), dtype=np.int64)
    for c in range(NCORES):
        ordv = np.argsort(-pvc[c], kind="stable")
        slot_of[c, ordv] = np.arange(VPAD)
        tb[c] = pvc[c][ordv].reshape(NBLK, BLK_V)[:, 0]

    core = vs // VLOC
    vloc_e = vs % VLOC
    s_e = slot_of[core, vloc_e]
    return {
        "order": order, "vglob": vs, "core": core,
        "blk": s_e // BLK_V, "lane": s_e % BLK_V,
        "pair": r // 2, "q": r & 1,
        "tb": tb, "slot_of": slot_of,
        "n_v": n_v, "starts": starts,
    }


def make_profile(tables):
    tb = np.stack([tab["tb"] for tab in tables])      # [T*? .. NCORES, NBLK]
    prof = tb.reshape(-1, NBLK).max(axis=0)
    return [max(int(x), 1) for x in prof]


def fill_stream(tab, features, adjacency, t, profile):
    """fp8 cascade-encode (4/deg folded) + scatter into per-core streams."""
    prof = np.asarray(profile, dtype=np.int64)
    colb = np.concatenate([[0], np.cumsum(prof * ROW_W)]).astype(np.int64)
    TOTW = int(colb[-1])

    adj = np.asarray(adjacency).reshape(V, T, N)
    deg = np.maximum((adj[:, t] >= 0).sum(axis=-1), 1).astype(np.float32)
    scale_v = (SCALE / deg).astype(np.float32)

    xs = np.asarray(features, dtype=np.float32)[tab["order"]]
    n_v, starts = tab["n_v"], tab["starts"]
    stored = np.empty((len(xs), F), dtype=np.uint8)
    vlist = np.nonzero(n_v > 0)[0]
    carry = np.zeros((len(vlist), F), dtype=np.float32)
    sc = scale_v[vlist][:, None]
    kv = n_v[vlist]
    st = starts[vlist]
    maxk = int(kv.max()) if len(kv) else 0
    for rank in range(maxk):
        m = kv > rank
        if not m.all():
            carry, sc, kv, st = carry[m], sc[m], kv[m], st[m]
        pos = st + rank
        y = xs[pos] * sc + carry
        q8 = y.astype(FP8)
        carry = y - q8.astype(np.float32)
        stored[pos] = q8.view(np.uint8)

    pos_e = (
        tab["core"] * (BLK_V * TOTW)
        + tab["lane"] * TOTW
        + colb[tab["blk"]] + tab["pair"] * ROW_W + tab["q"] * F
    )
    stream = np.zeros(NCORES * BLK_V * TOTW, dtype=np.uint8)
    stream[pos_e[:, None] + np.arange(F, dtype=np.int64)[None, :]] = stored
    return stream.reshape(NCORES, BLK_V, TOTW).view(FP8)


def build_device_program(profile, chunk=CHUNK):
    from contextlib import ExitStack

    import concourse.tile as tile
    from concourse import bacc, mybir

    f32 = mybir.dt.float32
    bf16 = mybir.dt.bfloat16
    f8 = mybir.dt.float8e4
    DR = mybir.MatmulPerfMode.DoubleRow

    prof = np.asarray(profile, dtype=np.int64)
    colb = np.concatenate([[0], np.cumsum(prof * ROW_W)]).astype(np.int64)
    TOTW = int(colb[-1])

    nc = bacc.Bacc("TRN2", target_bir_lowering=False, debug=False)
    feat_d = [
        nc.dram_tensor(f"feat{t}", [BLK_V, TOTW], f8, kind="ExternalInput").ap()
        for t in range(T)
    ]
    idd_d = nc.dram_tensor("idd", [128, 2 * BLK_V], f8, kind="ExternalInput").ap()
    out_d = nc.dram_tensor(
        "out", [T, NGRP, BLK_V, G * F], bf16, kind="ExternalOutput"
    ).ap()

    maxw = int(
        max(colb[min(i + chunk, NBLK)] - colb[i] for i in range(0, NBLK, chunk))
    )

    with tile.TileContext(nc) as tc, ExitStack() as ctx:
        const = ctx.enter_context(tc.tile_pool(name="const", bufs=1))
        featp = ctx.enter_context(tc.tile_pool(name="featp", bufs=6))
        outp = ctx.enter_context(tc.tile_pool(name="outp", bufs=3))
        psump = ctx.enter_context(tc.tile_pool(name="psum", bufs=1, space="PSUM"))

        idd_t = const.tile([128, 2 * BLK_V], f8)
        nc.gpsimd.dma_start(out=idd_t[:], in_=idd_d[:])
        lhs_dr = idd_t[:].rearrange("p (two v) -> p two v", two=2)
        lhs_n = idd_t[:, 0:BLK_V]

        # 8 persistent psum quad banks; block b -> bank b%8, slice (b//8)%4
        ps_banks = []
        for k in range(8):
            pb = psump.tile([BLK_V, 4 * BLK_V], f32, tag=f"ps{k}")
            ps_banks.append(pb)

        ndma = [0]
        for t in range(T):
            out_tiles = {}
            for ci in range(0, NBLK, chunk):
                ce = min(ci + chunk, NBLK)
                w = int(colb[ce] - colb[ci])
                ft = featp.tile([BLK_V, maxw], f8, tag="feat")
                ndma[0] += 1
                eng = nc.scalar if ndma[0] % 2 == 0 else nc.sync
                eng.dma_start(
                    out=ft[:, :w], in_=feat_d[t][:, int(colb[ci]):int(colb[ce])]
                )
                for b in range(ci, ce):
                    tbb = int(prof[b])
                    off = int(colb[b] - colb[ci])
                    g, so = divmod(b, G)
                    if so == 0:
                        ot = outp.tile([BLK_V, G * F], bf16, tag="outg")
                        out_tiles[g] = ot
                    pc = ((b // 8) % 4) * BLK_V
                    ps = ps_banks[b % 8][:, pc : pc + BLK_V]
                    # matmul plan: DR tile-pair chains into one 128-col psum
                    # slice of the quad bank, normal-mode single leftover tile
                    plan = []
                    i = 0
                    while tbb - i >= 2:
                        plan.append(("dr", i))
                        i += 2
                    if tbb - i == 1:
                        plan.append(("n", i))
                    for pi, (kind, i0) in enumerate(plan):
                        startf = pi == 0
                        stopf = pi == len(plan) - 1
                        if kind == "dr":
                            rhs = ft[
                                :, off + i0 * ROW_W : off + (i0 + 2) * ROW_W
                            ].rearrange("p (two n) -> p two n", two=2)
                            nc.tensor.matmul(
                                ps, lhsT=lhs_dr, rhs=rhs,
                                start=startf, stop=stopf, perf_mode=DR,
                            )
                        else:
                            nc.tensor.matmul(
                                ps, lhsT=lhs_n,
                                rhs=ft[:, off + i0 * ROW_W : off + (i0 + 1) * ROW_W],
                                start=startf, stop=stopf,
                            )
                    with nc.allow_low_precision(reason="bf16 mean output"):
                        nc.vector.tensor_reduce(
                            out_tiles[g][:, so * F : (so + 1) * F],
                            ps.rearrange("p (j f) -> p f j", f=F),
                            axis=mybir.AxisListType.X,
                            op=mybir.AluOpType.add,
                        )
                    if so == G - 1:
                        # the final group follows every feature chunk in
                        # program order, so HWDGE's lower first-byte latency
                        # shortens the tail without risking a FIFO stall
                        last = t == T - 1 and g == NGRP - 1
                        oeng = nc.sync if last else nc.gpsimd
                        oeng.dma_start(
                            out=out_d[t, g], in_=out_tiles.pop(g)[:]
                        )

    nc.compile()
    return nc


def prepare_inputs(adjacency, indices0, features0, indices1, features1):
    tab0 = shard_table(np.asarray(indices0))
    tab1 = shard_table(np.asarray(indices1))
    profile = make_profile([tab0, tab1])

    s0 = fill_stream(tab0, features0, adjacency, 0, profile)
    s1 = fill_stream(tab1, features1, adjacency, 1, profile)
    idd = np.concatenate(
        [np.eye(128, dtype=np.float32), np.eye(128, dtype=np.float32)], axis=1
    ).astype(FP8)

    in_maps = [
        {"feat0": s0[c], "feat1": s1[c], "idd": idd} for c in range(NCORES)
    ]
    meta = {"slot_of": [tab0["slot_of"], tab1["slot_of"]]}
    return in_maps, profile, meta


def assemble_output(core_outs, meta):
    outs = []
    for t in range(T):
        slot_of = meta["slot_of"][t]
        parts = []
        for c in range(NCORES):
            arr = np.asarray(core_outs[c]).astype(np.float32)[t]  # [NGRP,128,G*F]
            a = (
                arr.reshape(NGRP, BLK_V, G, F)
                .transpose(0, 2, 1, 3)
                .reshape(VPAD, F)
            )
            parts.append(a[slot_of[c, :VLOC]] * (1.0 / SCALE))
        outs.append(np.concatenate(parts, axis=0).reshape(B, V, F))
    return (outs[0], outs[1])


def kernel(adjacency, indices0, features0, indices1, features1):
    from concourse.bass_utils import run_bass_kernel_spmd

    in_maps, profile, meta = prepare_inputs(
        adjacency, indices0, features0, indices1, features1
    )

    key = tuple(profile)
    if key not in _NC_CACHE:
        _NC_CACHE[key] = build_device_program(profile)
    nc = _NC_CACHE[key]

    res = run_bass_kernel_spmd(nc, in_maps, list(range(NCORES)))
    return assemble_output(
        [res.results[c]["out"] for c in range(NCORES)], meta
    )


# revision 6
# speedup vs baseline: 1.0299x; 1.0299x over previous
"""Trainium2 Bass kernel for GNN mean aggregation (nn_AggrGSMean), v2.

Computes, for t in {0,1}:
    out_t[b, v, :] = segment_sum(features_t over edges with dest v) / degree[b, v, t]
where degree[b, v, t] = max(count(adjacency[b, v, t, :] >= 0), 1).

Strategy (graph-partition sharding; identity-weight redesign):
- Host: partition vertices across 8 cores; per (core, table) sort vertices by
  pair count (pairs = ceil(edges/2)) descending and chunk into 98 blocks of
  128.  Every vertex in block b is padded to exactly profile[b] pairs
  (profile = max tile count over cores x tables at each block rank, ~2%
  padding), so tile i of a block holds pair i of each of its 128 vertices in
  lane order: the matmul's stationary operand is a CONSTANT IDENTITY - no
  one-hot building on device at all.
- Features ship as fp8 e4m3 with 4/degree folded in, quantized with an
  error-feedback cascade per (vertex, column): each edge stores
  fp8(x + carry), so the device's exact f32 segment-sum recovers the true
  sum to within one final-carry (~0.7% rel err).  Host multiplies the bf16
  output by 0.25 (exact) during assembly.
- Device (per core): per table, feature stream [128, TOTW] fp8 arrives in
  ~8-block chunk DMAs (~1-2 MB each, alternating SP/ACT/GPSIMD rings).  Per
  block: DoubleRow fp8 matmuls (identity weights, groups of 8 tiles, N=512)
  accumulate pair-rows into a [128, 512] f32 PSUM bank at 2 fp8 MACs per
  cell per cycle; even remainders use a narrower DoubleRow, a final odd tile
  uses a normal-mode matmul (FWL).  One DVE tensor_reduce folds the psum
  column-groups + q-halves into the bf16 group output tile; out DMA per 7
  blocks.
"""

import sys

if "/opt/trn_rl_repo" not in sys.path:
    sys.path.insert(0, "/opt/trn_rl_repo")

import ml_dtypes
import numpy as np

# Problem constants (hardcoded per contract)
B, V, T, N, F, M = 1, 100000, 2, 32, 64, 1600000
NCORES = 8
BLK_V = 128                 # vertices per block == matmul output partitions
Q = 2                       # edges per pair-row
ROW_W = Q * F               # fp8 bytes per pair-row (128)
G = 7                       # blocks per output group
VLOC = V // NCORES          # 12500
VPAD = 12544                # padded vertices per core (98 * 128)
NBLK = VPAD // BLK_V        # 98
NGRP = NBLK // G            # 14
CHUNK = 4                   # blocks per feature DMA
SCALE = 4.0                 # power-of-2 pre-scale (undone exactly on host)

FP8 = ml_dtypes.float8_e4m3

_NC_CACHE = {}


def shard_table(indices):
    """Sort edges by destination; assign vertices to (block, lane) per core
    by descending pair count."""
    v = np.ascontiguousarray(indices[:, 1]).astype(np.int64)
    order = np.argsort(v, kind="stable")
    vs = v[order]
    n_v = np.bincount(vs, minlength=V)
    starts = np.concatenate([[0], np.cumsum(n_v)])
    r = np.arange(len(vs), dtype=np.int64) - starts[vs]
    pv = (n_v + 1) // 2

    pvc = np.zeros((NCORES, VPAD), dtype=np.int64)
    pvc[:, :VLOC] = pv.reshape(NCORES, VLOC)
    slot_of = np.empty((NCORES, VPAD), dtype=np.int64)
    tb = np.empty((NCORES, NBLK), dtype=np.int64)
    for c in range(NCORES):
        ordv = np.argsort(-pvc[c], kind="stable")
        slot_of[c, ordv] = np.arange(VPAD)
        tb[c] = pvc[c][ordv].reshape(NBLK, BLK_V)[:, 0]

    core = vs // VLOC
    vloc_e = vs % VLOC
    s_e = slot_of[core, vloc_e]
    return {
        "order": order, "vglob": vs, "core": core,
        "blk": s_e // BLK_V, "lane": s_e % BLK_V,
        "pair": r // 2, "q": r & 1,
        "tb": tb, "slot_of": slot_of,
        "n_v": n_v, "starts": starts,
    }


def make_profile(tables):
    tb = np.stack([tab["tb"] for tab in tables])      # [T*? .. NCORES, NBLK]
    prof = tb.reshape(-1, NBLK).max(axis=0)
    return [max(int(x), 1) for x in prof]


def fill_stream(tab, features, adjacency, t, profile):
    """fp8 cascade-encode (4/deg folded) + scatter into per-core streams."""
    prof = np.asarray(profile, dtype=np.int64)
    colb = np.concatenate([[0], np.cumsum(prof * ROW_W)]).astype(np.int64)
    TOTW = int(colb[-1])

    adj = np.asarray(adjacency).reshape(V, T, N)
    deg = np.maximum((adj[:, t] >= 0).sum(axis=-1), 1).astype(np.float32)
    scale_v = (SCALE / deg).astype(np.float32)

    xs = np.asarray(features, dtype=np.float32)[tab["order"]]
    n_v, starts = tab["n_v"], tab["starts"]
    stored = np.empty((len(xs), F), dtype=np.uint8)
    vlist = np.nonzero(n_v > 0)[0]
    carry = np.zeros((len(vlist), F), dtype=np.float32)
    sc = scale_v[vlist][:, None]
    kv = n_v[vlist]
    st = starts[vlist]
    maxk = int(kv.max()) if len(kv) else 0
    for rank in range(maxk):
        m = kv > rank
        if not m.all():
            carry, sc, kv, st = carry[m], sc[m], kv[m], st[m]
        pos = st + rank
        y = xs[pos] * sc + carry
        q8 = y.astype(FP8)
        carry = y - q8.astype(np.float32)
        stored[pos] = q8.view(np.uint8)

    pos_e = (
        tab["core"] * (BLK_V * TOTW)
        + tab["lane"] * TOTW
        + colb[tab["blk"]] + tab["pair"] * ROW_W + tab["q"] * F
    )
    stream = np.zeros(NCORES * BLK_V * TOTW, dtype=np.uint8)
    stream[pos_e[:, None] + np.arange(F, dtype=np.int64)[None, :]] = stored
    return stream.reshape(NCORES, BLK_V, TOTW).view(FP8)


def build_device_program(profile, chunk=CHUNK):
    from contextlib import ExitStack

    import concourse.tile as tile
    from concourse import bacc, mybir

    f32 = mybir.dt.float32
    bf16 = mybir.dt.bfloat16
    f8 = mybir.dt.float8e4
    DR = mybir.MatmulPerfMode.DoubleRow

    prof = np.asarray(profile, dtype=np.int64)
    colb = np.concatenate([[0], np.cumsum(prof * ROW_W)]).astype(np.int64)
    TOTW = int(colb[-1])

    nc = bacc.Bacc("TRN2", target_bir_lowering=False, debug=False)
    feat_d = [
        nc.dram_tensor(f"feat{t}", [BLK_V, TOTW], f8, kind="ExternalInput").ap()
        for t in range(T)
    ]
    idd_d = nc.dram_tensor("idd", [128, 2 * BLK_V], f8, kind="ExternalInput").ap()
    out_d = nc.dram_tensor(
        "out", [T, NGRP, BLK_V, G * F], bf16, kind="ExternalOutput"
    ).ap()

    maxw = int(
        max(colb[min(i + chunk, NBLK)] - colb[i] for i in range(0, NBLK, chunk))
    )

    with tile.TileContext(nc) as tc, ExitStack() as ctx:
        const = ctx.enter_context(tc.tile_pool(name="const", bufs=1))
        featp = ctx.enter_context(tc.tile_pool(name="featp", bufs=8))
        outp = ctx.enter_context(tc.tile_pool(name="outp", bufs=3))
        psump = ctx.enter_context(tc.tile_pool(name="psum", bufs=1, space="PSUM"))

        idd_t = const.tile([128, 2 * BLK_V], f8)
        nc.gpsimd.dma_start(out=idd_t[:], in_=idd_d[:])
        lhs_dr = idd_t[:].rearrange("p (two v) -> p two v", two=2)
        lhs_n = idd_t[:, 0:BLK_V]

        # 8 persistent psum quad banks; block b -> bank b%8, slice (b//8)%4
        ps_banks = []
        for k in range(8):
            pb = psump.tile([BLK_V, 4 * BLK_V], f32, tag=f"ps{k}")
            ps_banks.append(pb)

        ndma = [0]
        for t in range(T):
            out_tiles = {}
            for ci in range(0, NBLK, chunk):
                ce = min(ci + chunk, NBLK)
                w = int(colb[ce] - colb[ci])
                ft = featp.tile([BLK_V, maxw], f8, tag="feat")
                ndma[0] += 1
                eng = nc.scalar if ndma[0] % 2 == 0 else nc.sync
                eng.dma_start(
                    out=ft[:, :w], in_=feat_d[t][:, int(colb[ci]):int(colb[ce])]
                )
                for b in range(ci, ce):
                    tbb = int(prof[b])
                    off = int(colb[b] - colb[ci])
                    g, so = divmod(b, G)
                    if so == 0:
                        ot = outp.tile([BLK_V, G * F], bf16, tag="outg")
                        out_tiles[g] = ot
                    pc = ((b // 8) % 4) * BLK_V
                    ps = ps_banks[b % 8][:, pc : pc + BLK_V]
                    # matmul plan: DR tile-pair chains into one 128-col psum
                    # slice of the quad bank, normal-mode single leftover tile
                    plan = []
                    i = 0
                    while tbb - i >= 2:
                        plan.append(("dr", i))
                        i += 2
                    if tbb - i == 1:
                        plan.append(("n", i))
                    for pi, (kind, i0) in enumerate(plan):
                        startf = pi == 0
                        stopf = pi == len(plan) - 1
                        if kind == "dr":
                            rhs = ft[
                                :, off + i0 * ROW_W : off + (i0 + 2) * ROW_W
                            ].rearrange("p (two n) -> p two n", two=2)
                            nc.tensor.matmul(
                                ps, lhsT=lhs_dr, rhs=rhs,
                                start=startf, stop=stopf, perf_mode=DR,
                            )
                        else:
                            nc.tensor.matmul(
                                ps, lhsT=lhs_n,
                                rhs=ft[:, off + i0 * ROW_W : off + (i0 + 1) * ROW_W],
                                start=startf, stop=stopf,
                            )
                    with nc.allow_low_precision(reason="bf16 mean output"):
                        nc.vector.tensor_reduce(
                            out_tiles[g][:, so * F : (so + 1) * F],
                            ps.rearrange("p (j f) -> p f j", f=F),
                            axis=mybir.AxisListType.X,
                            op=mybir.AluOpType.add,
                        )
                    if so == G - 1:
                        # the final group follows every feature chunk in
                        # program order, so HWDGE's lower first-byte latency
                        # shortens the tail without risking a FIFO stall
                        last = t == T - 1 and g == NGRP - 1
                        oeng = nc.sync if last else nc.gpsimd
                        oeng.dma_start(
                            out=out_d[t, g], in_=out_tiles.pop(g)[:]
                        )

    nc.compile()
    return nc


def prepare_inputs(adjacency, indices0, features0, indices1, features1):
    tab0 = shard_table(np.asarray(indices0))
    tab1 = shard_table(np.asarray(indices1))
    profile = make_profile([tab0, tab1])

    s0 = fill_stream(tab0, features0, adjacency, 0, profile)
    s1 = fill_stream(tab1, features1, adjacency, 1, profile)
    idd = np.concatenate(
        [np.eye(128, dtype=np.float32), np.eye(128, dtype=np.float32)], axis=1
    ).astype(FP8)

    in_maps = [
        {"feat0": s0[c], "feat1": s1[c], "idd": idd} for c in range(NCORES)
    ]
    meta = {"slot_of": [tab0["slot_of"], tab1["slot_of"]]}
    return in_maps, profile, meta


def assemble_output(core_outs, meta):
    outs = []
    for t in range(T):
        slot_of = meta["slot_of"][t]
        parts = []
        for c in range(NCORES):
            arr = np.asarray(core_outs[c]).astype(np.float32)[t]  # [NGRP,128,G*F]
            a = (
                arr.reshape(NGRP, BLK_V, G, F)
                .transpose(0, 2, 1, 3)
                .reshape(VPAD, F)
            )
            parts.append(a[slot_of[c, :VLOC]] * (1.0 / SCALE))
        outs.append(np.concatenate(parts, axis=0).reshape(B, V, F))
    return (outs[0], outs[1])


def kernel(adjacency, indices0, features0, indices1, features1):
    from concourse.bass_utils import run_bass_kernel_spmd

    in_maps, profile, meta = prepare_inputs(
        adjacency, indices0, features0, indices1, features1
    )

    key = tuple(profile)
    if key not in _NC_CACHE:
        _NC_CACHE[key] = build_device_program(profile)
    nc = _NC_CACHE[key]

    res = run_bass_kernel_spmd(nc, in_maps, list(range(NCORES)))
    return assemble_output(
        [res.results[c]["out"] for c in range(NCORES)], meta
    )
